# revision 30
# baseline (speedup 1.0000x reference)
"""BigBird-style block-sparse attention on 8 Trainium2 NeuronCores.

Problem: B=2, H=12, S=4096, D=64, BLK=64 (64 blocks), R=3 random blocks.
All mask inputs are ones (per the generator spec); rand_attn drives the
gather structure and is read host-side.

Sharding: 24 (b,h) pairs -> 3 per core (data + head parallel).

Per-pair algorithm, all in "ST" layout (keys on PSUM partitions, queries on
the free axis):
  - middle blocks l=1..62 attend exactly 8 key blocks, organized as 4 packs
    of 128 keys: A=(l-1,l) [a kt slice], B=(l+1,r0), C=(r1,r2) [host
    gathered], G=(0,63) [shared; l=1/l=62 edge duplicates removed by
    zeroed V variants vg1/vg62, so no device-side masking at all].
  - l=0,63 attend densely to all keys (32 shared v packs).
  QK matmuls produce scores in PSUM, ACT does exp (scale and -2 bias fused;
  the bias cancels in the softmax ratio), PV matmuls contract keys with a
  ones-column appended to V so the denominator accumulates in out row 64.
  Output is the unnormalized ctx^T [65, 4096] bf16 per pair; the host
  divides by row 64 and transposes.

Emission is software-pipelined (QK of group g+1 issues before PV of group
g) so the PE never waits on the ACT engine and the DVFS clock stays high.
"""

import numpy as np

B, H, S, D = 2, 12, 4096, 64
BLK = 64
NB = S // BLK            # 64
R = 3
NPAIR = B * H            # 24
NCORE = 8
PPC = NPAIR // NCORE     # 3 pairs per core
NMID = 62                # l = 1..62
SCALE = 0.125            # 1/sqrt(64)
EBIAS = -2.0             # exp(s*SCALE + EBIAS): cancels in softmax ratio

# middle groups: 15 groups of 4 + 1 group of 2  (l = 1..62)
GROUPS = [(1 + 4 * g, 4) for g in range(15)] + [(61, 2)]

_COMPILED = {}


def _build_host_arrays(query_layer, key_layer, value_layer, rand_attn):
    import ml_dtypes
    bf16 = ml_dtypes.bfloat16

    q = np.ascontiguousarray(query_layer, dtype=np.float32).reshape(NPAIR, S, D)
    k = np.ascontiguousarray(key_layer, dtype=np.float32).reshape(NPAIR, S, D)
    v = np.ascontiguousarray(value_layer, dtype=np.float32).reshape(NPAIR, S, D)
    r = np.ascontiguousarray(rand_attn, dtype=np.int64).reshape(NPAIR, NMID, R)

    qt = np.ascontiguousarray(q.transpose(0, 2, 1)).astype(bf16)   # [24,64,S]
    kt = np.ascontiguousarray(k.transpose(0, 2, 1)).astype(bf16)   # [24,64,S]

    # dense q blocks {0, 63}: [24, 64, 128]
    qtd = np.concatenate([qt[:, :, 0:BLK], qt[:, :, S - BLK:]], axis=2)
    qtd = np.ascontiguousarray(qtd)
    # global key pack {0, 63}: [24, 64, 128]
    ktg = np.concatenate([kt[:, :, 0:BLK], kt[:, :, S - BLK:]], axis=2)
    ktg = np.ascontiguousarray(ktg)

    # kr: per-l gathered packs B=(l+1, r0), C=(r1, r2): [24, 64, 62*256]
    kb = kt.reshape(NPAIR, D, NB, BLK)                # [24, 64, 64, 64]
    bh = np.arange(NPAIR)[:, None, None]
    ls = np.arange(1, NMID + 1)                       # l = 1..62
    blocks = np.empty((NPAIR, NMID, 4), np.int64)
    blocks[:, :, 0] = ls[None, :] + 1                 # l+1
    blocks[:, :, 1:] = r                              # r0, r1, r2
    kr = kb[bh, :, blocks]                            # -> [24, 62, 4, 64, 64]? check
    # fancy index: kb[bh(24,1,1), :, blocks(24,62,4)] -> [24, 62, 4, 64, 64]
    kr = np.ascontiguousarray(kr.transpose(0, 3, 1, 2, 4)
                              .reshape(NPAIR, D, NMID * 4 * BLK))

    ones = np.ones((NPAIR, NB, BLK, 1), np.float32)
    v65 = np.concatenate([v.reshape(NPAIR, NB, BLK, D), ones], axis=3)  # [24,64,64,65]

    # vw: all consecutive-pair v packs j=0..62: keys j*64 .. j*64+128
    # [24, 63, 128, 65] -> [24, 128, 63*65]
    v65f = v65.reshape(NPAIR, NB * BLK, D + 1)
    idx = (np.arange(63)[:, None] * BLK + np.arange(128)[None, :])      # [63,128]
    vw = v65f[:, idx]                                 # [24, 63, 128, 65]
    vw = np.ascontiguousarray(vw.transpose(0, 2, 1, 3)
                              .reshape(NPAIR, 128, 63 * (D + 1))).astype(bf16)

    # vr: per-l packs B=(v_{l+1}, v_{r0}), C=(v_{r1}, v_{r2}):
    # [24, 62, 4, 64, 65] -> pairs -> [24, 128, 62*2*65]
    vr = v65[bh, blocks]                              # [24, 62, 4, 64, 65]
    vr = vr.reshape(NPAIR, NMID, 2, 2, BLK, D + 1)    # [24,62,2pack,2half,64,65]
    vr = vr.reshape(NPAIR, NMID, 2, 128, D + 1)
    vr = np.ascontiguousarray(vr.transpose(0, 3, 1, 2, 4)
                              .reshape(NPAIR, 128, NMID * 2 * (D + 1))).astype(bf16)

    # global v pack {0, 63} + edge variants
    vg_full = np.concatenate([v65[:, 0], v65[:, NB - 1]], axis=1)  # [24,128,65]
    vg1 = vg_full.copy()
    vg1[:, 0:BLK, :] = 0.0        # l=1: block 0 already in its window pack A
    vg62 = vg_full.copy()
    vg62[:, BLK:, :] = 0.0        # l=62: block 63 already in its pack B
    vg = np.ascontiguousarray(vg_full).astype(bf16)
    vg1 = np.ascontiguousarray(vg1).astype(bf16)
    vg62 = np.ascontiguousarray(vg62).astype(bf16)

    kq = np.concatenate([qtd, ktg], axis=2).astype(bf16)       # [24, 64, 256]
    vgx = np.concatenate([vg, vg1, vg62], axis=2)               # [24, 128, 195]
    return dict(qt=qt, kt=kt, kq=kq, kr=kr.astype(bf16), vw=vw, vr=vr,
                vgx=np.ascontiguousarray(vgx))


def _fixup_multiwait(nc, mybir):
    """Split >1-sem-wait instructions (the Tile exit drain) into single-wait
    NoOps: this walrus build's CTRL codegen has one wait slot."""
    for fn in nc.m.functions:
        for bb in fn.blocks:
            insts = list(bb.instructions)
            out = []
            for inst in insts:
                si = inst.sync_info
                if si is not None and len(si.on_wait) > 1:
                    waits = list(si.on_wait)
                    for kk, w in enumerate(waits[:-1]):
                        nop = mybir.InstNoOp(
                            name=f"{inst.name}-wsplit{kk}",
                            opcode="NoOp",
                            engine=inst.engine,
                            sync_info=mybir.SyncInfo(on_wait=[w], on_update=[]),
                        )
                        out.append(nop)
                    si.on_wait = [waits[-1]]
                    inst.sync_info = si
                out.append(inst)
            bb.instructions = out


def _build_program(apply_fixup=True):
    import sys
    if "/opt/trn_rl_repo" not in sys.path:
        sys.path.insert(0, "/opt/trn_rl_repo")
    import concourse.bass as bass
    import concourse.mybir as mybir
    from concourse.tile import TileContext

    f32 = mybir.dt.float32
    bf16 = mybir.dt.bfloat16
    EXP = mybir.ActivationFunctionType.Exp

    nc = bass.Bass("TRN2", target_bir_lowering=False, debug=False,
                   num_devices=NCORE)

    # register a const AP for the exp bias
    _bias_t = nc.alloc_sbuf_tensor("const-f32-ebias", [128, 1], f32)
    nc.gpsimd.memset(_bias_t.ap(), EBIAS)
    nc.const_aps.aps[(f32, EBIAS)] = _bias_t.ap()
    nc.all_engine_barrier()

    d_qt = nc.dram_tensor("qt", [PPC, D, S], bf16, kind="ExternalInput").ap()
    d_kt = nc.dram_tensor("kt", [PPC, D, S], bf16, kind="ExternalInput").ap()
    d_kq = nc.dram_tensor("kq", [PPC, D, 256], bf16, kind="ExternalInput").ap()
    d_kr = nc.dram_tensor("kr", [PPC, D, NMID * 256], bf16, kind="ExternalInput").ap()
    d_vw = nc.dram_tensor("vw", [PPC, 128, 63 * 65], bf16, kind="ExternalInput").ap()
    d_vr = nc.dram_tensor("vr", [PPC, 128, NMID * 2 * 65], bf16, kind="ExternalInput").ap()
    d_vgx = nc.dram_tensor("vgx", [PPC, 128, 195], bf16, kind="ExternalInput").ap()
    d_out = nc.dram_tensor("out", [PPC, 65, S], bf16, kind="ExternalOutput").ap()

    with TileContext(nc) as tc:
        with tc.tile_pool(name="sb", bufs=2) as sb, \
             tc.tile_pool(name="ps", bufs=2, space="PSUM") as ps, \
             tc.tile_pool(name="ptp", bufs=6) as ptp, \
             tc.tile_pool(name="aux", bufs=2) as aux:

            # warm the Exp activation table while input DMAs stream
            warm = aux.tile([128, 1], f32, name="warm", tag="warm", bufs=1)
            nc.scalar.activation(warm, nc.const_aps.aps[(f32, 0.0)][:, 0:1],
                                 EXP, scale=1.0)

            # static double-buffered K/Q tensors: rows 64:128 are zeroed
            # exactly once per buffer half, then never dirtied (DMA fills
            # rows 0:64 only), so all matmuls contract K=128
            kq2 = nc.alloc_sbuf_tensor("kq2", [128, 512], bf16).ap()
            kt2 = nc.alloc_sbuf_tensor("kt2", [128, 2 * S], bf16).ap()
            qt2 = nc.alloc_sbuf_tensor("qt2", [128, 2 * S], bf16).ap()
            kr2 = nc.alloc_sbuf_tensor("kr2", [128, 2 * NMID * 256], bf16).ap()

            def zero_half(eng, h):
                # need-order: dense wants kq+kt, early groups want qt+kr head
                eng.memset(kq2[64:128, h * 256:(h + 1) * 256], 0.0)
                for c_ in range(4):
                    eng.memset(kt2[64:128, h * S + c_ * 1024:
                                    h * S + (c_ + 1) * 1024], 0.0)
                base = h * NMID * 256
                eng.memset(qt2[64:128, h * S:h * S + 1024], 0.0)
                for c_ in range(4):
                    eng.memset(kr2[64:128, base + c_ * 1984:
                                    base + (c_ + 1) * 1984], 0.0)
                eng.memset(qt2[64:128, h * S + 1024:h * S + 2048], 0.0)
                for c_ in range(4, 8):
                    eng.memset(kr2[64:128, base + c_ * 1984:
                                    base + (c_ + 1) * 1984], 0.0)
                eng.memset(qt2[64:128, h * S + 2048:h * S + 3072], 0.0)
                eng.memset(qt2[64:128, h * S + 3072:h * S + 4096], 0.0)

            zero_half(nc.vector, 0)

            pend = []          # global software pipeline, depth 2

            for p in range(PPC):
                # K/Q tiles are 128 partitions tall: rows 0:64 carry data,
                # rows 64:128 are zeros so every matmul contracts K=128 (the
                # tensor engine only clocks up under full-K contractions).
                # Zeros are written once per pool buffer (pairs 0/1) and
                # persist across buffer reuse.
                h = p % 2
                qt = qt2[:, h * S:(h + 1) * S]
                kt = kt2[:, h * S:(h + 1) * S]
                kq = kq2[:, h * 256:(h + 1) * 256]
                kr = kr2[:, h * NMID * 256:(h + 1) * NMID * 256]
                vw = sb.tile([128, 63 * 65], bf16, name=f"vw{p}", tag="vw",
                             bufs=3)
                vr = sb.tile([128, NMID * 2 * 65], bf16, name=f"vr{p}", tag="vr",
                             bufs=3)
                vgx = sb.tile([128, 195], bf16, name=f"vgx{p}", tag="vgx")
                qtd = kq[:, 0:128]
                ktg = kq[:, 128:256]

                # sync queue (HW DGE), need-ordered; pair-0's dense-
                # critical kq/kt ride the idle Activation queue so the first
                # matmul's lumped DMA-sem wait covers only those 3 transfers
                headq = nc.sync
                headq.dma_start(out=kq[0:64, :], in_=d_kq[p])
                headq.dma_start(out=kt[0:64, 0:1024], in_=d_kt[p][:, 0:1024])
                headq.dma_start(out=kt[0:64, 1024:], in_=d_kt[p][:, 1024:])
                hv = 32 * 65
                h_qt = S // 2
                nc.sync.dma_start(out=vw[:, 0:hv], in_=d_vw[p][:, 0:hv])
                nc.sync.dma_start(out=qt[0:64, 0:h_qt], in_=d_qt[p][:, 0:h_qt])
                nc.sync.dma_start(out=vgx, in_=d_vgx[p])
                nc.sync.dma_start(out=vw[:, hv:], in_=d_vw[p][:, hv:])
                nc.sync.dma_start(out=qt[0:64, h_qt:], in_=d_qt[p][:, h_qt:])
                h_vr = NMID * 65
                nc.sync.dma_start(out=vr[:, 0:h_vr], in_=d_vr[p][:, 0:h_vr])
                nc.sync.dma_start(out=vr[:, h_vr:], in_=d_vr[p][:, h_vr:])
                # gpsimd queue (SW DGE): rand K packs
                h_kr = NMID * 128
                nc.gpsimd.dma_start(out=kr[0:64, 0:h_kr], in_=d_kr[p][:, 0:h_kr])
                nc.gpsimd.dma_start(out=kr[0:64, h_kr:], in_=d_kr[p][:, h_kr:])

                if p == 0:
                    # zero buffer half 1 on the Pool engine, after pair-0's
                    # kr DGE configs so they are not delayed
                    zero_half(nc.gpsimd, 1)

                ostage = aux.tile([65, S], bf16, name=f"ostage{p}", tag="os")

                def mk(p, qt, kt, kq, kr, vw, vr, vgx, qtd, ktg, ostage):
                    def vw_pack(j):
                        return vw[:, j * 65:(j + 1) * 65]

                    def dense_qk(c):
                        st = ps.tile([128, 1024], f32, name=f"std{p}_{c}",
                                     tag="st", bufs=3)
                        for j in range(8):
                            nc.tensor.matmul(
                                st[:, j * 128:(j + 1) * 128],
                                lhsT=kt[:, (8 * c + j) * 128:(8 * c + j + 1) * 128],
                                rhs=qtd, start=True, stop=True)
                        pt = ptp.tile([128, 1024], bf16, name=f"ptd{p}_{c}",
                                      tag="pt")
                        nc.scalar.activation(pt, st, EXP, scale=SCALE, bias=EBIAS)
                        return st, pt

                    def dense_pv(c, st_pt, ctxd):
                        st, pt = st_pt
                        for j in range(8):
                            nc.tensor.matmul(
                                ctxd[0:65, 0:128],
                                lhsT=vw_pack(2 * (8 * c + j)),
                                rhs=pt[:, j * 128:(j + 1) * 128],
                                start=(c == 0 and j == 0),
                                stop=(c == 3 and j == 7))

                    def group_qk(g):
                        l0, nl = GROUPS[g]
                        g0 = nl * 192
                        st = ps.tile([128, 1024], f32, name=f"st{p}_{g}",
                                     tag="st", bufs=3)
                        for j in range(nl):
                            l = l0 + j
                            o = j * 192
                            rhs = qt[:, l * BLK:(l + 1) * BLK]
                            nc.tensor.matmul(
                                st[:, o:o + 64],
                                lhsT=kt[:, (l - 1) * BLK:(l + 1) * BLK],
                                rhs=rhs, start=True, stop=True)
                            kb_ = kr[:, (l - 1) * 256:(l - 1) * 256 + 128]
                            kc_ = kr[:, (l - 1) * 256 + 128:(l - 1) * 256 + 256]
                            nc.tensor.matmul(st[:, o + 64:o + 128], lhsT=kb_,
                                             rhs=rhs, start=True, stop=True)
                            nc.tensor.matmul(st[:, o + 128:o + 192], lhsT=kc_,
                                             rhs=rhs, start=True, stop=True)
                        nc.tensor.matmul(st[:, g0:g0 + nl * 64], lhsT=ktg,
                                         rhs=qt[:, l0 * BLK:(l0 + nl) * BLK],
                                         start=True, stop=True)
                        pt = ptp.tile([128, 1024], bf16, name=f"pt{p}_{g}",
                                      tag="pt")
                        nc.scalar.activation(pt[:, 0:nl * 256],
                                             st[:, 0:nl * 256],
                                             EXP, scale=SCALE, bias=EBIAS)
                        return st, pt

                    def group_pv(g, st_pt, ctx):
                        st, pt = st_pt
                        l0, nl = GROUPS[g]
                        g0 = nl * 192
                        for j in range(nl):
                            l = l0 + j
                            o = j * 192
                            oc = j * BLK
                            vb_ = vr[:, (l - 1) * 130:(l - 1) * 130 + 65]
                            vc_ = vr[:, (l - 1) * 130 + 65:(l - 1) * 130 + 130]
                            nc.tensor.matmul(ctx[0:65, oc:oc + 64],
                                             lhsT=vw_pack(l - 1),
                                             rhs=pt[:, o:o + 64],
                                             start=True, stop=False)
                            nc.tensor.matmul(ctx[0:65, oc:oc + 64], lhsT=vb_,
                                             rhs=pt[:, o + 64:o + 128],
                                             start=False, stop=False)
                            nc.tensor.matmul(ctx[0:65, oc:oc + 64], lhsT=vc_,
                                             rhs=pt[:, o + 128:o + 192],
                                             start=False, stop=False)
                            vg_ = (vgx[:, 65:130] if l == 1 else
                                   vgx[:, 130:195] if l == 62 else
                                   vgx[:, 0:65])
                            nc.tensor.matmul(
                                ctx[0:65, oc:oc + 64],
                                lhsT=vg_,
                                rhs=pt[:, g0 + j * 64:g0 + (j + 1) * 64],
                                start=False, stop=True)

                    def group_out(g, ctx):
                        l0, nl = GROUPS[g]
                        w = nl * BLK
                        nc.vector.tensor_copy(
                            ostage[:, l0 * BLK:l0 * BLK + w], ctx[0:65, 0:w])

                    ctx_of = {}
                    ctxd_box = []

                    def run_qk(s):
                        kind, i = s
                        if kind == "d":
                            if i == 0:
                                ctxd_box.append(ps.tile([128, 128], f32,
                                                        name=f"ctxd{p}",
                                                        tag="ctx"))
                            return dense_qk(i)
                        ctx_of[i] = ps.tile([128, 256], f32,
                                            name=f"ctx{p}_{i}", tag="ctx")
                        return group_qk(i)

                    def run_pv(s, st_pt):
                        kind, i = s
                        if kind == "d":
                            dense_pv(i, st_pt, ctxd_box[0])
                            if i == 3:
                                nc.vector.tensor_copy(ostage[:, 0:BLK],
                                                      ctxd_box[0][0:65, 0:64])
                                nc.vector.tensor_copy(ostage[:, S - BLK:],
                                                      ctxd_box[0][0:65, 64:128])
                        else:
                            group_pv(i, st_pt, ctx_of[i])
                            group_out(i, ctx_of[i])
                            if i == len(GROUPS) - 1:
                                nc.sync.dma_start(out=d_out[p][0:33, :],
                                                  in_=ostage[0:33, :])
                                nc.sync.dma_start(out=d_out[p][33:65, :],
                                                  in_=ostage[33:65, :])
                    return run_qk, run_pv

                run_qk, run_pv = mk(p, qt, kt, kq, kr, vw, vr, vgx, qtd, ktg,
                                    ostage)
                for s in ([("d", c) for c in range(4)] +
                          [("g", g) for g in range(len(GROUPS))]):
                    pend.append((run_pv, s, run_qk(s)))
                    if len(pend) > 2:
                        fpv, ps_, st_ = pend.pop(0)
                        fpv(ps_, st_)
            for fpv, ps_, st_ in pend:
                fpv(ps_, st_)

    if apply_fixup:
        _fixup_multiwait(nc, mybir)
    return nc


def _get_program():
    if "nc" not in _COMPILED:
        _COMPILED["nc"] = _build_program()
    return _COMPILED["nc"]


def kernel(query_layer, key_layer, value_layer, band_mask, from_mask, to_mask,
           from_blocked_mask, to_blocked_mask, rand_attn):
    import sys
    if "/opt/trn_rl_repo" not in sys.path:
        sys.path.insert(0, "/opt/trn_rl_repo")
    from concourse.bass_utils import run_bass_kernel_spmd

    arrs = _build_host_arrays(query_layer, key_layer, value_layer, rand_attn)
    nc = _get_program()

    in_maps = []
    for c in range(NCORE):
        sl = slice(c * PPC, (c + 1) * PPC)
        in_maps.append({k: np.ascontiguousarray(v[sl]) for k, v in arrs.items()})

    res = run_bass_kernel_spmd(nc, in_maps, list(range(NCORE)))

    outs = np.stack([np.asarray(res.results[c]["out"]) for c in range(NCORE)])
    outs = outs.reshape(NPAIR, 65, S).astype(np.float64)
    ctx = outs[:, :64, :] / outs[:, 64:65, :]                        # [24,64,S]
    ctx = ctx.transpose(0, 2, 1).reshape(B, H, S, D)                 # [B,H,S,D]
    out = ctx.transpose(0, 2, 1, 3).astype(np.float32)               # [B,S,H,D]
    return np.ascontiguousarray(out)


# revision 31
# speedup vs baseline: 1.0025x; 1.0025x over previous
"""BigBird-style block-sparse attention on 8 Trainium2 NeuronCores.

Problem: B=2, H=12, S=4096, D=64, BLK=64 (64 blocks), R=3 random blocks.
All mask inputs are ones (per the generator spec); rand_attn drives the
gather structure and is read host-side.

Sharding: 24 (b,h) pairs -> 3 per core (data + head parallel).

Per-pair algorithm, all in "ST" layout (keys on PSUM partitions, queries on
the free axis):
  - middle blocks l=1..62 attend exactly 8 key blocks, organized as 4 packs
    of 128 keys: A=(l-1,l) [a kt slice], B=(l+1,r0), C=(r1,r2) [host
    gathered], G=(0,63) [shared; l=1/l=62 edge duplicates removed by
    zeroed V variants vg1/vg62, so no device-side masking at all].
  - l=0,63 attend densely to all keys (32 shared v packs).
  QK matmuls produce scores in PSUM, ACT does exp (scale and -2 bias fused;
  the bias cancels in the softmax ratio), PV matmuls contract keys with a
  ones-column appended to V so the denominator accumulates in out row 64.
  Output is the unnormalized ctx^T [65, 4096] bf16 per pair; the host
  divides by row 64 and transposes.

Emission is software-pipelined (QK of group g+1 issues before PV of group
g) so the PE never waits on the ACT engine and the DVFS clock stays high.
"""

import numpy as np

B, H, S, D = 2, 12, 4096, 64
BLK = 64
NB = S // BLK            # 64
R = 3
NPAIR = B * H            # 24
NCORE = 8
PPC = NPAIR // NCORE     # 3 pairs per core
NMID = 62                # l = 1..62
SCALE = 0.125            # 1/sqrt(64)
EBIAS = -2.0             # exp(s*SCALE + EBIAS): cancels in softmax ratio

# middle groups: 15 groups of 4 + 1 group of 2  (l = 1..62)
GROUPS = [(1 + 4 * g, 4) for g in range(15)] + [(61, 2)]

_COMPILED = {}


def _build_host_arrays(query_layer, key_layer, value_layer, rand_attn):
    import ml_dtypes
    bf16 = ml_dtypes.bfloat16

    q = np.ascontiguousarray(query_layer, dtype=np.float32).reshape(NPAIR, S, D)
    k = np.ascontiguousarray(key_layer, dtype=np.float32).reshape(NPAIR, S, D)
    v = np.ascontiguousarray(value_layer, dtype=np.float32).reshape(NPAIR, S, D)
    r = np.ascontiguousarray(rand_attn, dtype=np.int64).reshape(NPAIR, NMID, R)

    qt = np.ascontiguousarray(q.transpose(0, 2, 1)).astype(bf16)   # [24,64,S]
    kt = np.ascontiguousarray(k.transpose(0, 2, 1)).astype(bf16)   # [24,64,S]

    # dense q blocks {0, 63}: [24, 64, 128]
    qtd = np.concatenate([qt[:, :, 0:BLK], qt[:, :, S - BLK:]], axis=2)
    qtd = np.ascontiguousarray(qtd)
    # global key pack {0, 63}: [24, 64, 128]
    ktg = np.concatenate([kt[:, :, 0:BLK], kt[:, :, S - BLK:]], axis=2)
    ktg = np.ascontiguousarray(ktg)

    # kr: per-l gathered packs B=(l+1, r0), C=(r1, r2): [24, 64, 62*256]
    kb = kt.reshape(NPAIR, D, NB, BLK)                # [24, 64, 64, 64]
    bh = np.arange(NPAIR)[:, None, None]
    ls = np.arange(1, NMID + 1)                       # l = 1..62
    blocks = np.empty((NPAIR, NMID, 4), np.int64)
    blocks[:, :, 0] = ls[None, :] + 1                 # l+1
    blocks[:, :, 1:] = r                              # r0, r1, r2
    kr = kb[bh, :, blocks]                            # -> [24, 62, 4, 64, 64]? check
    # fancy index: kb[bh(24,1,1), :, blocks(24,62,4)] -> [24, 62, 4, 64, 64]
    kr = np.ascontiguousarray(kr.transpose(0, 3, 1, 2, 4)
                              .reshape(NPAIR, D, NMID * 4 * BLK))

    ones = np.ones((NPAIR, NB, BLK, 1), np.float32)
    v65 = np.concatenate([v.reshape(NPAIR, NB, BLK, D), ones], axis=3)  # [24,64,64,65]

    # vw: all consecutive-pair v packs j=0..62: keys j*64 .. j*64+128
    # [24, 63, 128, 65] -> [24, 128, 63*65]
    v65f = v65.reshape(NPAIR, NB * BLK, D + 1)
    idx = (np.arange(63)[:, None] * BLK + np.arange(128)[None, :])      # [63,128]
    vw = v65f[:, idx]                                 # [24, 63, 128, 65]
    vw = np.ascontiguousarray(vw.transpose(0, 2, 1, 3)
                              .reshape(NPAIR, 128, 63 * (D + 1))).astype(bf16)

    # vr: per-l packs B=(v_{l+1}, v_{r0}), C=(v_{r1}, v_{r2}):
    # [24, 62, 4, 64, 65] -> pairs -> [24, 128, 62*2*65]
    vr = v65[bh, blocks]                              # [24, 62, 4, 64, 65]
    vr = vr.reshape(NPAIR, NMID, 2, 2, BLK, D + 1)    # [24,62,2pack,2half,64,65]
    vr = vr.reshape(NPAIR, NMID, 2, 128, D + 1)
    vr = np.ascontiguousarray(vr.transpose(0, 3, 1, 2, 4)
                              .reshape(NPAIR, 128, NMID * 2 * (D + 1))).astype(bf16)

    # global v pack {0, 63} + edge variants
    vg_full = np.concatenate([v65[:, 0], v65[:, NB - 1]], axis=1)  # [24,128,65]
    vg1 = vg_full.copy()
    vg1[:, 0:BLK, :] = 0.0        # l=1: block 0 already in its window pack A
    vg62 = vg_full.copy()
    vg62[:, BLK:, :] = 0.0        # l=62: block 63 already in its pack B
    vg = np.ascontiguousarray(vg_full).astype(bf16)
    vg1 = np.ascontiguousarray(vg1).astype(bf16)
    vg62 = np.ascontiguousarray(vg62).astype(bf16)

    kq = np.concatenate([qtd, ktg], axis=2).astype(bf16)       # [24, 64, 256]
    vgx = np.concatenate([vg, vg1, vg62], axis=2)               # [24, 128, 195]
    return dict(qt=qt, kt=kt, kq=kq, kr=kr.astype(bf16), vw=vw, vr=vr,
                vgx=np.ascontiguousarray(vgx))


def _fixup_multiwait(nc, mybir):
    """Split >1-sem-wait instructions (the Tile exit drain) into single-wait
    NoOps: this walrus build's CTRL codegen has one wait slot."""
    for fn in nc.m.functions:
        for bb in fn.blocks:
            insts = list(bb.instructions)
            out = []
            for inst in insts:
                si = inst.sync_info
                if si is not None and len(si.on_wait) > 1:
                    waits = list(si.on_wait)
                    for kk, w in enumerate(waits[:-1]):
                        nop = mybir.InstNoOp(
                            name=f"{inst.name}-wsplit{kk}",
                            opcode="NoOp",
                            engine=inst.engine,
                            sync_info=mybir.SyncInfo(on_wait=[w], on_update=[]),
                        )
                        out.append(nop)
                    si.on_wait = [waits[-1]]
                    inst.sync_info = si
                out.append(inst)
            bb.instructions = out


def _build_program(apply_fixup=True):
    import sys
    if "/opt/trn_rl_repo" not in sys.path:
        sys.path.insert(0, "/opt/trn_rl_repo")
    import concourse.bass as bass
    import concourse.mybir as mybir
    from concourse.tile import TileContext

    f32 = mybir.dt.float32
    bf16 = mybir.dt.bfloat16
    EXP = mybir.ActivationFunctionType.Exp

    nc = bass.Bass("TRN2", target_bir_lowering=False, debug=False,
                   num_devices=NCORE)

    # register a const AP for the exp bias
    _bias_t = nc.alloc_sbuf_tensor("const-f32-ebias", [128, 1], f32)
    nc.gpsimd.memset(_bias_t.ap(), EBIAS)
    nc.const_aps.aps[(f32, EBIAS)] = _bias_t.ap()
    nc.all_engine_barrier()

    d_qt = nc.dram_tensor("qt", [PPC, D, S], bf16, kind="ExternalInput").ap()
    d_kt = nc.dram_tensor("kt", [PPC, D, S], bf16, kind="ExternalInput").ap()
    d_kq = nc.dram_tensor("kq", [PPC, D, 256], bf16, kind="ExternalInput").ap()
    d_kr = nc.dram_tensor("kr", [PPC, D, NMID * 256], bf16, kind="ExternalInput").ap()
    d_vw = nc.dram_tensor("vw", [PPC, 128, 63 * 65], bf16, kind="ExternalInput").ap()
    d_vr = nc.dram_tensor("vr", [PPC, 128, NMID * 2 * 65], bf16, kind="ExternalInput").ap()
    d_vgx = nc.dram_tensor("vgx", [PPC, 128, 195], bf16, kind="ExternalInput").ap()
    d_out = nc.dram_tensor("out", [PPC, 65, S], bf16, kind="ExternalOutput").ap()

    with TileContext(nc) as tc:
        with tc.tile_pool(name="sb", bufs=2) as sb, \
             tc.tile_pool(name="ps", bufs=2, space="PSUM") as ps, \
             tc.tile_pool(name="ptp", bufs=4) as ptp, \
             tc.tile_pool(name="aux", bufs=2) as aux:

            # warm the Exp activation table while input DMAs stream
            warm = aux.tile([128, 1], f32, name="warm", tag="warm", bufs=1)
            nc.scalar.activation(warm, nc.const_aps.aps[(f32, 0.0)][:, 0:1],
                                 EXP, scale=1.0)

            # static double-buffered K/Q tensors: rows 64:128 are zeroed
            # exactly once per buffer half, then never dirtied (DMA fills
            # rows 0:64 only), so all matmuls contract K=128
            kq2 = nc.alloc_sbuf_tensor("kq2", [128, 512], bf16).ap()
            kt2 = nc.alloc_sbuf_tensor("kt2", [128, 2 * S], bf16).ap()
            qt2 = nc.alloc_sbuf_tensor("qt2", [128, 2 * S], bf16).ap()
            kr2 = nc.alloc_sbuf_tensor("kr2", [128, 2 * NMID * 256], bf16).ap()

            def zero_half(eng, h):
                # need-order: dense wants kq+kt, early groups want qt+kr head
                eng.memset(kq2[64:128, h * 256:(h + 1) * 256], 0.0)
                for c_ in range(4):
                    eng.memset(kt2[64:128, h * S + c_ * 1024:
                                    h * S + (c_ + 1) * 1024], 0.0)
                base = h * NMID * 256
                eng.memset(qt2[64:128, h * S:h * S + 1024], 0.0)
                for c_ in range(4):
                    eng.memset(kr2[64:128, base + c_ * 1984:
                                    base + (c_ + 1) * 1984], 0.0)
                eng.memset(qt2[64:128, h * S + 1024:h * S + 2048], 0.0)
                for c_ in range(4, 8):
                    eng.memset(kr2[64:128, base + c_ * 1984:
                                    base + (c_ + 1) * 1984], 0.0)
                eng.memset(qt2[64:128, h * S + 2048:h * S + 3072], 0.0)
                eng.memset(qt2[64:128, h * S + 3072:h * S + 4096], 0.0)

            zero_half(nc.vector, 0)

            pend = []          # global software pipeline, depth 2

            for p in range(PPC):
                # K/Q tiles are 128 partitions tall: rows 0:64 carry data,
                # rows 64:128 are zeros so every matmul contracts K=128 (the
                # tensor engine only clocks up under full-K contractions).
                # Zeros are written once per pool buffer (pairs 0/1) and
                # persist across buffer reuse.
                h = p % 2
                qt = qt2[:, h * S:(h + 1) * S]
                kt = kt2[:, h * S:(h + 1) * S]
                kq = kq2[:, h * 256:(h + 1) * 256]
                kr = kr2[:, h * NMID * 256:(h + 1) * NMID * 256]
                vw = sb.tile([128, 63 * 65], bf16, name=f"vw{p}", tag="vw",
                             bufs=3)
                vr = sb.tile([128, NMID * 2 * 65], bf16, name=f"vr{p}", tag="vr",
                             bufs=3)
                vgx = sb.tile([128, 195], bf16, name=f"vgx{p}", tag="vgx")
                qtd = kq[:, 0:128]
                ktg = kq[:, 128:256]

                # sync queue (HW DGE), need-ordered; pair-0's dense-
                # critical kq/kt ride the idle Activation queue so the first
                # matmul's lumped DMA-sem wait covers only those 3 transfers
                headq = nc.sync
                headq.dma_start(out=kq[0:64, :], in_=d_kq[p])
                headq.dma_start(out=kt[0:64, 0:1024], in_=d_kt[p][:, 0:1024])
                headq.dma_start(out=kt[0:64, 1024:], in_=d_kt[p][:, 1024:])
                hv = 32 * 65
                h_qt = S // 2
                nc.sync.dma_start(out=vw[:, 0:hv], in_=d_vw[p][:, 0:hv])
                nc.sync.dma_start(out=qt[0:64, 0:h_qt], in_=d_qt[p][:, 0:h_qt])
                nc.sync.dma_start(out=vgx, in_=d_vgx[p])
                nc.sync.dma_start(out=vw[:, hv:], in_=d_vw[p][:, hv:])
                nc.sync.dma_start(out=qt[0:64, h_qt:], in_=d_qt[p][:, h_qt:])
                h_vr = NMID * 65
                nc.sync.dma_start(out=vr[:, 0:h_vr], in_=d_vr[p][:, 0:h_vr])
                nc.sync.dma_start(out=vr[:, h_vr:], in_=d_vr[p][:, h_vr:])
                # gpsimd queue (SW DGE): rand K packs
                h_kr = NMID * 128
                nc.gpsimd.dma_start(out=kr[0:64, 0:h_kr], in_=d_kr[p][:, 0:h_kr])
                nc.gpsimd.dma_start(out=kr[0:64, h_kr:], in_=d_kr[p][:, h_kr:])

                if p == 0:
                    # zero buffer half 1 on the Pool engine, after pair-0's
                    # kr DGE configs so they are not delayed
                    zero_half(nc.gpsimd, 1)

                ostage = aux.tile([65, S], bf16, name=f"ostage{p}", tag="os")

                def mk(p, qt, kt, kq, kr, vw, vr, vgx, qtd, ktg, ostage):
                    def vw_pack(j):
                        return vw[:, j * 65:(j + 1) * 65]

                    def dense_qk(c):
                        st = ps.tile([128, 1024], f32, name=f"std{p}_{c}",
                                     tag="st", bufs=3)
                        for j in range(8):
                            nc.tensor.matmul(
                                st[:, j * 128:(j + 1) * 128],
                                lhsT=kt[:, (8 * c + j) * 128:(8 * c + j + 1) * 128],
                                rhs=qtd, start=True, stop=True)
                        pt = ptp.tile([128, 1024], bf16, name=f"ptd{p}_{c}",
                                      tag="pt")
                        nc.scalar.activation(pt, st, EXP, scale=SCALE, bias=EBIAS)
                        return st, pt

                    def dense_pv(c, st_pt, ctxd):
                        st, pt = st_pt
                        for j in range(8):
                            nc.tensor.matmul(
                                ctxd[0:65, 0:128],
                                lhsT=vw_pack(2 * (8 * c + j)),
                                rhs=pt[:, j * 128:(j + 1) * 128],
                                start=(c == 0 and j == 0),
                                stop=(c == 3 and j == 7))

                    def group_qk(g):
                        l0, nl = GROUPS[g]
                        g0 = nl * 192
                        st = ps.tile([128, 1024], f32, name=f"st{p}_{g}",
                                     tag="st", bufs=3)
                        for j in range(nl):
                            l = l0 + j
                            o = j * 192
                            rhs = qt[:, l * BLK:(l + 1) * BLK]
                            nc.tensor.matmul(
                                st[:, o:o + 64],
                                lhsT=kt[:, (l - 1) * BLK:(l + 1) * BLK],
                                rhs=rhs, start=True, stop=True)
                            kb_ = kr[:, (l - 1) * 256:(l - 1) * 256 + 128]
                            kc_ = kr[:, (l - 1) * 256 + 128:(l - 1) * 256 + 256]
                            nc.tensor.matmul(st[:, o + 64:o + 128], lhsT=kb_,
                                             rhs=rhs, start=True, stop=True)
                            nc.tensor.matmul(st[:, o + 128:o + 192], lhsT=kc_,
                                             rhs=rhs, start=True, stop=True)
                        nc.tensor.matmul(st[:, g0:g0 + nl * 64], lhsT=ktg,
                                         rhs=qt[:, l0 * BLK:(l0 + nl) * BLK],
                                         start=True, stop=True)
                        pt = ptp.tile([128, 1024], bf16, name=f"pt{p}_{g}",
                                      tag="pt")
                        nc.scalar.activation(pt[:, 0:nl * 256],
                                             st[:, 0:nl * 256],
                                             EXP, scale=SCALE, bias=EBIAS)
                        return st, pt

                    def group_pv(g, st_pt, ctx):
                        st, pt = st_pt
                        l0, nl = GROUPS[g]
                        g0 = nl * 192
                        for j in range(nl):
                            l = l0 + j
                            o = j * 192
                            oc = j * BLK
                            vb_ = vr[:, (l - 1) * 130:(l - 1) * 130 + 65]
                            vc_ = vr[:, (l - 1) * 130 + 65:(l - 1) * 130 + 130]
                            nc.tensor.matmul(ctx[0:65, oc:oc + 64],
                                             lhsT=vw_pack(l - 1),
                                             rhs=pt[:, o:o + 64],
                                             start=True, stop=False)
                            nc.tensor.matmul(ctx[0:65, oc:oc + 64], lhsT=vb_,
                                             rhs=pt[:, o + 64:o + 128],
                                             start=False, stop=False)
                            nc.tensor.matmul(ctx[0:65, oc:oc + 64], lhsT=vc_,
                                             rhs=pt[:, o + 128:o + 192],
                                             start=False, stop=False)
                            vg_ = (vgx[:, 65:130] if l == 1 else
                                   vgx[:, 130:195] if l == 62 else
                                   vgx[:, 0:65])
                            nc.tensor.matmul(
                                ctx[0:65, oc:oc + 64],
                                lhsT=vg_,
                                rhs=pt[:, g0 + j * 64:g0 + (j + 1) * 64],
                                start=False, stop=True)

                    def group_out(g, ctx):
                        l0, nl = GROUPS[g]
                        w = nl * BLK
                        nc.vector.tensor_copy(
                            ostage[:, l0 * BLK:l0 * BLK + w], ctx[0:65, 0:w])

                    ctx_of = {}
                    ctxd_box = []

                    def run_qk(s):
                        kind, i = s
                        if kind == "d":
                            if i == 0:
                                ctxd_box.append(ps.tile([128, 128], f32,
                                                        name=f"ctxd{p}",
                                                        tag="ctx"))
                            return dense_qk(i)
                        ctx_of[i] = ps.tile([128, 256], f32,
                                            name=f"ctx{p}_{i}", tag="ctx")
                        return group_qk(i)

                    def run_pv(s, st_pt):
                        kind, i = s
                        if kind == "d":
                            dense_pv(i, st_pt, ctxd_box[0])
                            if i == 3:
                                nc.vector.tensor_copy(ostage[:, 0:BLK],
                                                      ctxd_box[0][0:65, 0:64])
                                nc.vector.tensor_copy(ostage[:, S - BLK:],
                                                      ctxd_box[0][0:65, 64:128])
                        else:
                            group_pv(i, st_pt, ctx_of[i])
                            group_out(i, ctx_of[i])
                            if i == len(GROUPS) - 1:
                                nc.sync.dma_start(out=d_out[p][0:33, :],
                                                  in_=ostage[0:33, :])
                                nc.sync.dma_start(out=d_out[p][33:65, :],
                                                  in_=ostage[33:65, :])
                    return run_qk, run_pv

                run_qk, run_pv = mk(p, qt, kt, kq, kr, vw, vr, vgx, qtd, ktg,
                                    ostage)
                for s in ([("d", c) for c in range(4)] +
                          [("g", g) for g in range(len(GROUPS))]):
                    pend.append((run_pv, s, run_qk(s)))
                    if len(pend) > 2:
                        fpv, ps_, st_ = pend.pop(0)
                        fpv(ps_, st_)
            for fpv, ps_, st_ in pend:
                fpv(ps_, st_)

    if apply_fixup:
        _fixup_multiwait(nc, mybir)
    return nc


def _get_program():
    if "nc" not in _COMPILED:
        _COMPILED["nc"] = _build_program()
    return _COMPILED["nc"]


def kernel(query_layer, key_layer, value_layer, band_mask, from_mask, to_mask,
           from_blocked_mask, to_blocked_mask, rand_attn):
    import sys
    if "/opt/trn_rl_repo" not in sys.path:
        sys.path.insert(0, "/opt/trn_rl_repo")
    from concourse.bass_utils import run_bass_kernel_spmd

    arrs = _build_host_arrays(query_layer, key_layer, value_layer, rand_attn)
    nc = _get_program()

    in_maps = []
    for c in range(NCORE):
        sl = slice(c * PPC, (c + 1) * PPC)
        in_maps.append({k: np.ascontiguousarray(v[sl]) for k, v in arrs.items()})

    res = run_bass_kernel_spmd(nc, in_maps, list(range(NCORE)))

    outs = np.stack([np.asarray(res.results[c]["out"]) for c in range(NCORE)])
    outs = outs.reshape(NPAIR, 65, S).astype(np.float64)
    ctx = outs[:, :64, :] / outs[:, 64:65, :]                        # [24,64,S]
    ctx = ctx.transpose(0, 2, 1).reshape(B, H, S, D)                 # [B,H,S,D]
    out = ctx.transpose(0, 2, 1, 3).astype(np.float32)               # [B,S,H,D]
    return np.ascontiguousarray(out)


# revision 32
# speedup vs baseline: 1.0126x; 1.0100x over previous
"""BigBird-style block-sparse attention on 8 Trainium2 NeuronCores.

Problem: B=2, H=12, S=4096, D=64, BLK=64 (64 blocks), R=3 random blocks.
All mask inputs are ones (per the generator spec); rand_attn drives the
gather structure and is read host-side.

Sharding: 24 (b,h) pairs -> 3 per core (data + head parallel).

Per-pair algorithm, all in "ST" layout (keys on PSUM partitions, queries on
the free axis):
  - middle blocks l=1..62 attend exactly 8 key blocks, organized as 4 packs
    of 128 keys: A=(l-1,l) [a kt slice], B=(l+1,r0), C=(r1,r2) [host
    gathered], G=(0,63) [shared; l=1/l=62 edge duplicates removed by
    zeroed V variants vg1/vg62, so no device-side masking at all].
  - l=0,63 attend densely to all keys (32 shared v packs).
  QK matmuls produce scores in PSUM, ACT does exp (scale and -2 bias fused;
  the bias cancels in the softmax ratio), PV matmuls contract keys with a
  ones-column appended to V so the denominator accumulates in out row 64.
  Output is the unnormalized ctx^T [65, 4096] bf16 per pair; the host
  divides by row 64 and transposes.

Emission is software-pipelined (QK of group g+1 issues before PV of group
g) so the PE never waits on the ACT engine and the DVFS clock stays high.
"""

import numpy as np

B, H, S, D = 2, 12, 4096, 64
BLK = 64
NB = S // BLK            # 64
R = 3
NPAIR = B * H            # 24
NCORE = 8
PPC = NPAIR // NCORE     # 3 pairs per core
NMID = 62                # l = 1..62
SCALE = 0.125            # 1/sqrt(64)
EBIAS = -2.0             # exp(s*SCALE + EBIAS): cancels in softmax ratio

# middle groups: 15 groups of 4 + 1 group of 2  (l = 1..62)
GROUPS = [(1 + 4 * g, 4) for g in range(15)] + [(61, 2)]

_COMPILED = {}


def _build_host_arrays(query_layer, key_layer, value_layer, rand_attn):
    import ml_dtypes
    bf16 = ml_dtypes.bfloat16

    q = np.ascontiguousarray(query_layer, dtype=np.float32).reshape(NPAIR, S, D)
    k = np.ascontiguousarray(key_layer, dtype=np.float32).reshape(NPAIR, S, D)
    v = np.ascontiguousarray(value_layer, dtype=np.float32).reshape(NPAIR, S, D)
    r = np.ascontiguousarray(rand_attn, dtype=np.int64).reshape(NPAIR, NMID, R)

    qt = np.ascontiguousarray(q.transpose(0, 2, 1)).astype(bf16)   # [24,64,S]
    kt = np.ascontiguousarray(k.transpose(0, 2, 1)).astype(bf16)   # [24,64,S]

    # dense q blocks {0, 63}: [24, 64, 128]
    qtd = np.concatenate([qt[:, :, 0:BLK], qt[:, :, S - BLK:]], axis=2)
    qtd = np.ascontiguousarray(qtd)
    # global key pack {0, 63}: [24, 64, 128]
    ktg = np.concatenate([kt[:, :, 0:BLK], kt[:, :, S - BLK:]], axis=2)
    ktg = np.ascontiguousarray(ktg)

    # kr: per-l gathered packs B=(l+1, r0), C=(r1, r2): [24, 64, 62*256]
    kb = kt.reshape(NPAIR, D, NB, BLK)                # [24, 64, 64, 64]
    bh = np.arange(NPAIR)[:, None, None]
    ls = np.arange(1, NMID + 1)                       # l = 1..62
    blocks = np.empty((NPAIR, NMID, 4), np.int64)
    blocks[:, :, 0] = ls[None, :] + 1                 # l+1
    blocks[:, :, 1:] = r                              # r0, r1, r2
    kr = kb[bh, :, blocks]                            # -> [24, 62, 4, 64, 64]? check
    # fancy index: kb[bh(24,1,1), :, blocks(24,62,4)] -> [24, 62, 4, 64, 64]
    kr = np.ascontiguousarray(kr.transpose(0, 3, 1, 2, 4)
                              .reshape(NPAIR, D, NMID * 4 * BLK))

    ones = np.ones((NPAIR, NB, BLK, 1), np.float32)
    v65 = np.concatenate([v.reshape(NPAIR, NB, BLK, D), ones], axis=3)  # [24,64,64,65]

    # vw: all consecutive-pair v packs j=0..62: keys j*64 .. j*64+128
    # [24, 63, 128, 65] -> [24, 128, 63*65]
    v65f = v65.reshape(NPAIR, NB * BLK, D + 1)
    idx = (np.arange(63)[:, None] * BLK + np.arange(128)[None, :])      # [63,128]
    vw = v65f[:, idx]                                 # [24, 63, 128, 65]
    vw = np.ascontiguousarray(vw.transpose(0, 2, 1, 3)
                              .reshape(NPAIR, 128, 63 * (D + 1))).astype(bf16)

    # vr: per-l packs B=(v_{l+1}, v_{r0}), C=(v_{r1}, v_{r2}):
    # [24, 62, 4, 64, 65] -> pairs -> [24, 128, 62*2*65]
    vr = v65[bh, blocks]                              # [24, 62, 4, 64, 65]
    vr = vr.reshape(NPAIR, NMID, 2, 2, BLK, D + 1)    # [24,62,2pack,2half,64,65]
    vr = vr.reshape(NPAIR, NMID, 2, 128, D + 1)
    vr = np.ascontiguousarray(vr.transpose(0, 3, 1, 2, 4)
                              .reshape(NPAIR, 128, NMID * 2 * (D + 1))).astype(bf16)

    # global v pack {0, 63} + edge variants
    vg_full = np.concatenate([v65[:, 0], v65[:, NB - 1]], axis=1)  # [24,128,65]
    vg1 = vg_full.copy()
    vg1[:, 0:BLK, :] = 0.0        # l=1: block 0 already in its window pack A
    vg62 = vg_full.copy()
    vg62[:, BLK:, :] = 0.0        # l=62: block 63 already in its pack B
    vg = np.ascontiguousarray(vg_full).astype(bf16)
    vg1 = np.ascontiguousarray(vg1).astype(bf16)
    vg62 = np.ascontiguousarray(vg62).astype(bf16)

    kq = np.concatenate([qtd, ktg], axis=2).astype(bf16)       # [24, 64, 256]
    vgx = np.concatenate([vg, vg1, vg62], axis=2)               # [24, 128, 195]
    return dict(qt=qt, kt=kt, kq=kq, kr=kr.astype(bf16), vw=vw, vr=vr,
                vgx=np.ascontiguousarray(vgx))


def _fixup_multiwait(nc, mybir):
    """Split >1-sem-wait instructions (the Tile exit drain) into single-wait
    NoOps: this walrus build's CTRL codegen has one wait slot."""
    for fn in nc.m.functions:
        for bb in fn.blocks:
            insts = list(bb.instructions)
            out = []
            for inst in insts:
                si = inst.sync_info
                if si is not None and len(si.on_wait) > 1:
                    waits = list(si.on_wait)
                    for kk, w in enumerate(waits[:-1]):
                        nop = mybir.InstNoOp(
                            name=f"{inst.name}-wsplit{kk}",
                            opcode="NoOp",
                            engine=inst.engine,
                            sync_info=mybir.SyncInfo(on_wait=[w], on_update=[]),
                        )
                        out.append(nop)
                    si.on_wait = [waits[-1]]
                    inst.sync_info = si
                out.append(inst)
            bb.instructions = out


def _build_program(apply_fixup=True):
    import sys
    if "/opt/trn_rl_repo" not in sys.path:
        sys.path.insert(0, "/opt/trn_rl_repo")
    import concourse.bass as bass
    import concourse.mybir as mybir
    from concourse.tile import TileContext

    f32 = mybir.dt.float32
    bf16 = mybir.dt.bfloat16
    EXP = mybir.ActivationFunctionType.Exp

    nc = bass.Bass("TRN2", target_bir_lowering=False, debug=False,
                   num_devices=NCORE)

    # register a const AP for the exp bias
    _bias_t = nc.alloc_sbuf_tensor("const-f32-ebias", [128, 1], f32)
    nc.gpsimd.memset(_bias_t.ap(), EBIAS)
    nc.const_aps.aps[(f32, EBIAS)] = _bias_t.ap()
    nc.all_engine_barrier()

    d_qt = nc.dram_tensor("qt", [PPC, D, S], bf16, kind="ExternalInput").ap()
    d_kt = nc.dram_tensor("kt", [PPC, D, S], bf16, kind="ExternalInput").ap()
    d_kq = nc.dram_tensor("kq", [PPC, D, 256], bf16, kind="ExternalInput").ap()
    d_kr = nc.dram_tensor("kr", [PPC, D, NMID * 256], bf16, kind="ExternalInput").ap()
    d_vw = nc.dram_tensor("vw", [PPC, 128, 63 * 65], bf16, kind="ExternalInput").ap()
    d_vr = nc.dram_tensor("vr", [PPC, 128, NMID * 2 * 65], bf16, kind="ExternalInput").ap()
    d_vgx = nc.dram_tensor("vgx", [PPC, 128, 195], bf16, kind="ExternalInput").ap()
    d_out = nc.dram_tensor("out", [PPC, 65, S], bf16, kind="ExternalOutput").ap()

    with TileContext(nc) as tc:
        with tc.tile_pool(name="sb", bufs=2) as sb, \
             tc.tile_pool(name="ps", bufs=2, space="PSUM") as ps, \
             tc.tile_pool(name="ptp", bufs=4) as ptp, \
             tc.tile_pool(name="aux", bufs=2) as aux:

            # warm the Exp activation table while input DMAs stream
            warm = aux.tile([128, 1], f32, name="warm", tag="warm", bufs=1)
            nc.scalar.activation(warm, nc.const_aps.aps[(f32, 0.0)][:, 0:1],
                                 EXP, scale=1.0)

            # static double-buffered K/Q tensors: rows 64:128 are zeroed
            # exactly once per buffer half, then never dirtied (DMA fills
            # rows 0:64 only), so all matmuls contract K=128
            kq2 = nc.alloc_sbuf_tensor("kq2", [128, 512], bf16).ap()
            kt2 = nc.alloc_sbuf_tensor("kt2", [128, 2 * S], bf16).ap()
            qt2 = nc.alloc_sbuf_tensor("qt2", [128, 2 * S], bf16).ap()
            kr2 = nc.alloc_sbuf_tensor("kr2", [128, 2 * NMID * 256], bf16).ap()

            def zero_half(eng, h):
                # need-order: dense wants kq+kt, early groups want qt+kr head
                eng.memset(kq2[64:128, h * 256:(h + 1) * 256], 0.0)
                for c_ in range(4):
                    eng.memset(kt2[64:128, h * S + c_ * 1024:
                                    h * S + (c_ + 1) * 1024], 0.0)
                base = h * NMID * 256
                eng.memset(qt2[64:128, h * S:h * S + 1024], 0.0)
                for c_ in range(4):
                    eng.memset(kr2[64:128, base + c_ * 1984:
                                    base + (c_ + 1) * 1984], 0.0)
                eng.memset(qt2[64:128, h * S + 1024:h * S + 2048], 0.0)
                for c_ in range(4, 8):
                    eng.memset(kr2[64:128, base + c_ * 1984:
                                    base + (c_ + 1) * 1984], 0.0)
                eng.memset(qt2[64:128, h * S + 2048:h * S + 3072], 0.0)
                eng.memset(qt2[64:128, h * S + 3072:h * S + 4096], 0.0)

            zero_half(nc.vector, 0)

            pend = []          # global software pipeline, depth 2

            for p in range(PPC):
                # K/Q tiles are 128 partitions tall: rows 0:64 carry data,
                # rows 64:128 are zeros so every matmul contracts K=128 (the
                # tensor engine only clocks up under full-K contractions).
                # Zeros are written once per pool buffer (pairs 0/1) and
                # persist across buffer reuse.
                h = p % 2
                qt = qt2[:, h * S:(h + 1) * S]
                kt = kt2[:, h * S:(h + 1) * S]
                kq = kq2[:, h * 256:(h + 1) * 256]
                kr = kr2[:, h * NMID * 256:(h + 1) * NMID * 256]
                vw = sb.tile([128, 63 * 65], bf16, name=f"vw{p}", tag="vw",
                             bufs=3)
                vr = sb.tile([128, NMID * 2 * 65], bf16, name=f"vr{p}", tag="vr",
                             bufs=3)
                vgx = sb.tile([128, 195], bf16, name=f"vgx{p}", tag="vgx")
                qtd = kq[:, 0:128]
                ktg = kq[:, 128:256]

                # sync queue (HW DGE), need-ordered. For pair 0 the
                # emission of later transfers is deferred into the stage
                # loop: the Tile DMA-sem wait of a consumer is lumped to
                # 2x(#starts emitted so far on that queue), so late emission
                # keeps the first matmuls gated only on what they need.
                hv = 32 * 65
                h_qt = S // 2
                h_vr = NMID * 65
                dma_plan = [
                    (None, lambda: nc.sync.dma_start(out=kq[0:64, :],
                                                     in_=d_kq[p])),
                    (None, lambda: nc.sync.dma_start(
                        out=kt[0:64, 0:1024], in_=d_kt[p][:, 0:1024])),
                    (("d", 0), lambda: nc.sync.dma_start(
                        out=kt[0:64, 1024:], in_=d_kt[p][:, 1024:])),
                    (("d", 0), lambda: nc.sync.dma_start(
                        out=vw[:, 0:hv], in_=d_vw[p][:, 0:hv])),
                    (("d", 1), lambda: nc.sync.dma_start(
                        out=qt[0:64, 0:h_qt], in_=d_qt[p][:, 0:h_qt])),
                    (("d", 1), lambda: nc.sync.dma_start(out=vgx,
                                                         in_=d_vgx[p])),
                    (("d", 2), lambda: nc.sync.dma_start(
                        out=vw[:, hv:], in_=d_vw[p][:, hv:])),
                    (("d", 2), lambda: nc.sync.dma_start(
                        out=qt[0:64, h_qt:], in_=d_qt[p][:, h_qt:])),
                    (("d", 3), lambda: nc.sync.dma_start(
                        out=vr[:, 0:h_vr], in_=d_vr[p][:, 0:h_vr])),
                    (("g", 1), lambda: nc.sync.dma_start(
                        out=vr[:, h_vr:], in_=d_vr[p][:, h_vr:])),
                ]
                if p == 0:
                    deferred = [(k, f) for k, f in dma_plan if k is not None]
                    for k, f in dma_plan:
                        if k is None:
                            f()
                else:
                    deferred = []
                    for _, f in dma_plan:
                        f()
                # gpsimd queue (SW DGE): rand K packs
                h_kr = NMID * 128
                nc.gpsimd.dma_start(out=kr[0:64, 0:h_kr], in_=d_kr[p][:, 0:h_kr])
                nc.gpsimd.dma_start(out=kr[0:64, h_kr:], in_=d_kr[p][:, h_kr:])

                if p == 0:
                    # zero buffer half 1 on the Pool engine, after pair-0's
                    # kr DGE configs so they are not delayed
                    zero_half(nc.gpsimd, 1)

                ostage = aux.tile([65, S], bf16, name=f"ostage{p}", tag="os")

                def mk(p, qt, kt, kq, kr, vw, vr, vgx, qtd, ktg, ostage):
                    def vw_pack(j):
                        return vw[:, j * 65:(j + 1) * 65]

                    def dense_qk(c):
                        st = ps.tile([128, 1024], f32, name=f"std{p}_{c}",
                                     tag="st", bufs=3)
                        for j in range(8):
                            nc.tensor.matmul(
                                st[:, j * 128:(j + 1) * 128],
                                lhsT=kt[:, (8 * c + j) * 128:(8 * c + j + 1) * 128],
                                rhs=qtd, start=True, stop=True)
                        pt = ptp.tile([128, 1024], bf16, name=f"ptd{p}_{c}",
                                      tag="pt")
                        nc.scalar.activation(pt, st, EXP, scale=SCALE, bias=EBIAS)
                        return st, pt

                    def dense_pv(c, st_pt, ctxd):
                        st, pt = st_pt
                        for j in range(8):
                            nc.tensor.matmul(
                                ctxd[0:65, 0:128],
                                lhsT=vw_pack(2 * (8 * c + j)),
                                rhs=pt[:, j * 128:(j + 1) * 128],
                                start=(c == 0 and j == 0),
                                stop=(c == 3 and j == 7))

                    def group_qk(g):
                        l0, nl = GROUPS[g]
                        g0 = nl * 192
                        st = ps.tile([128, 1024], f32, name=f"st{p}_{g}",
                                     tag="st", bufs=3)
                        for j in range(nl):
                            l = l0 + j
                            o = j * 192
                            rhs = qt[:, l * BLK:(l + 1) * BLK]
                            nc.tensor.matmul(
                                st[:, o:o + 64],
                                lhsT=kt[:, (l - 1) * BLK:(l + 1) * BLK],
                                rhs=rhs, start=True, stop=True)
                            kb_ = kr[:, (l - 1) * 256:(l - 1) * 256 + 128]
                            kc_ = kr[:, (l - 1) * 256 + 128:(l - 1) * 256 + 256]
                            nc.tensor.matmul(st[:, o + 64:o + 128], lhsT=kb_,
                                             rhs=rhs, start=True, stop=True)
                            nc.tensor.matmul(st[:, o + 128:o + 192], lhsT=kc_,
                                             rhs=rhs, start=True, stop=True)
                        nc.tensor.matmul(st[:, g0:g0 + nl * 64], lhsT=ktg,
                                         rhs=qt[:, l0 * BLK:(l0 + nl) * BLK],
                                         start=True, stop=True)
                        pt = ptp.tile([128, 1024], bf16, name=f"pt{p}_{g}",
                                      tag="pt")
                        nc.scalar.activation(pt[:, 0:nl * 256],
                                             st[:, 0:nl * 256],
                                             EXP, scale=SCALE, bias=EBIAS)
                        return st, pt

                    def group_pv(g, st_pt, ctx):
                        st, pt = st_pt
                        l0, nl = GROUPS[g]
                        g0 = nl * 192
                        for j in range(nl):
                            l = l0 + j
                            o = j * 192
                            oc = j * BLK
                            vb_ = vr[:, (l - 1) * 130:(l - 1) * 130 + 65]
                            vc_ = vr[:, (l - 1) * 130 + 65:(l - 1) * 130 + 130]
                            nc.tensor.matmul(ctx[0:65, oc:oc + 64],
                                             lhsT=vw_pack(l - 1),
                                             rhs=pt[:, o:o + 64],
                                             start=True, stop=False)
                            nc.tensor.matmul(ctx[0:65, oc:oc + 64], lhsT=vb_,
                                             rhs=pt[:, o + 64:o + 128],
                                             start=False, stop=False)
                            nc.tensor.matmul(ctx[0:65, oc:oc + 64], lhsT=vc_,
                                             rhs=pt[:, o + 128:o + 192],
                                             start=False, stop=False)
                            vg_ = (vgx[:, 65:130] if l == 1 else
                                   vgx[:, 130:195] if l == 62 else
                                   vgx[:, 0:65])
                            nc.tensor.matmul(
                                ctx[0:65, oc:oc + 64],
                                lhsT=vg_,
                                rhs=pt[:, g0 + j * 64:g0 + (j + 1) * 64],
                                start=False, stop=True)

                    def group_out(g, ctx):
                        l0, nl = GROUPS[g]
                        w = nl * BLK
                        nc.vector.tensor_copy(
                            ostage[:, l0 * BLK:l0 * BLK + w], ctx[0:65, 0:w])

                    ctx_of = {}
                    ctxd_box = []

                    def run_qk(s):
                        kind, i = s
                        if kind == "d":
                            if i == 0:
                                ctxd_box.append(ps.tile([128, 128], f32,
                                                        name=f"ctxd{p}",
                                                        tag="ctx"))
                            return dense_qk(i)
                        ctx_of[i] = ps.tile([128, 256], f32,
                                            name=f"ctx{p}_{i}", tag="ctx")
                        return group_qk(i)

                    def run_pv(s, st_pt):
                        kind, i = s
                        if kind == "d":
                            dense_pv(i, st_pt, ctxd_box[0])
                            if i == 3:
                                nc.vector.tensor_copy(ostage[:, 0:BLK],
                                                      ctxd_box[0][0:65, 0:64])
                                nc.vector.tensor_copy(ostage[:, S - BLK:],
                                                      ctxd_box[0][0:65, 64:128])
                        else:
                            group_pv(i, st_pt, ctx_of[i])
                            group_out(i, ctx_of[i])
                            if i == len(GROUPS) - 1:
                                nc.sync.dma_start(out=d_out[p][0:33, :],
                                                  in_=ostage[0:33, :])
                                nc.sync.dma_start(out=d_out[p][33:65, :],
                                                  in_=ostage[33:65, :])
                    return run_qk, run_pv

                run_qk, run_pv = mk(p, qt, kt, kq, kr, vw, vr, vgx, qtd, ktg,
                                    ostage)
                for s in ([("d", c) for c in range(4)] +
                          [("g", g) for g in range(len(GROUPS))]):
                    pend.append((run_pv, s, run_qk(s)))
                    while deferred and deferred[0][0] == s:
                        deferred.pop(0)[1]()
                    if len(pend) > 2:
                        fpv, ps_, st_ = pend.pop(0)
                        fpv(ps_, st_)
            for fpv, ps_, st_ in pend:
                fpv(ps_, st_)

    if apply_fixup:
        _fixup_multiwait(nc, mybir)
    return nc


def _get_program():
    if "nc" not in _COMPILED:
        _COMPILED["nc"] = _build_program()
    return _COMPILED["nc"]


def kernel(query_layer, key_layer, value_layer, band_mask, from_mask, to_mask,
           from_blocked_mask, to_blocked_mask, rand_attn):
    import sys
    if "/opt/trn_rl_repo" not in sys.path:
        sys.path.insert(0, "/opt/trn_rl_repo")
    from concourse.bass_utils import run_bass_kernel_spmd

    arrs = _build_host_arrays(query_layer, key_layer, value_layer, rand_attn)
    nc = _get_program()

    in_maps = []
    for c in range(NCORE):
        sl = slice(c * PPC, (c + 1) * PPC)
        in_maps.append({k: np.ascontiguousarray(v[sl]) for k, v in arrs.items()})

    res = run_bass_kernel_spmd(nc, in_maps, list(range(NCORE)))

    outs = np.stack([np.asarray(res.results[c]["out"]) for c in range(NCORE)])
    outs = outs.reshape(NPAIR, 65, S).astype(np.float64)
    ctx = outs[:, :64, :] / outs[:, 64:65, :]                        # [24,64,S]
    ctx = ctx.transpose(0, 2, 1).reshape(B, H, S, D)                 # [B,H,S,D]
    out = ctx.transpose(0, 2, 1, 3).astype(np.float32)               # [B,S,H,D]
    return np.ascontiguousarray(out)
